# revision 32
# baseline (speedup 1.0000x reference)
"""Trainium2 Bass kernel: multi-head self-attention block (dense transformer).

Reference computation (fp32):
    qkv = x @ w_qkv + b_qkv                  # x [b, n, dim], w_qkv [dim, 3*dim]
    q, k, v = split(qkv); heads = 16, dh = 64
    dots = (q @ k^T) * dim**-0.5  (per head)
    attn = softmax(dots, axis=-1)
    out  = (attn @ v) @ w_out + b_out        # [b, n, dim]

Sharding (8 cores): data-parallel over batch (b=2) x tensor-parallel over
head-groups (4 groups of 4 heads).  core c -> batch c//4, head-group c%4.
Each core computes q/k/v for its 4 heads only, runs attention, and multiplies
by its 256-row slice of w_out, producing a partial [n, dim] output.  The host
sums the 4 partials per batch (the "all-reduce") and adds b_out.

v2 design (bf16 + engine-balance; ~2x over the fp32r v1):
  - ALL matmul operands are bf16 (fp32 PSUM accumulate).  fp32r pays an
    unhidden ~107ns LDWEIGHTS per matmul (no FWL for fp32); bf16 enables
    FWL so an N=512 matmul costs ~216ns vs ~330ns fp32r.  End-to-end bf16
    keeps global rel err ~5e-3 (gate 2e-2).
  - the scalar engine (ACT) runs exp at 1 elem/cycle/lane @1.2GHz: 16.8M
    score elements/core = ~110us busy -- a hard floor that rivals the PE
    (~140us).  So: ACT does ONLY exp (one FD=1024 instruction per key-tile,
    both heads of the pair in one [128,2,512] PSUM tile); every PSUM->SBUF
    copy runs on DVE; normalization uses reciprocal_approx_fast (5x faster
    than DVE reciprocal, 18-bit accurate) once per (chunk, pair).
  - software pipeline ACROSS the whole kernel, not per-phase: after the
    k-projection and first q-chunk land, scores+exp stream continuously;
    v-projection, remaining q-projection and the output projection are
    drip-fed into the PE stream as filler so the PE never idles while ACT
    works; attn@v trails exp by one (chunk, pair) unit (deep esb buffer).
  - scores are computed TRANSPOSED (S^T [j, i]) so attn@v needs no
    transpose; attn@v stationary = [v_h | ones] (M=128): rows 0-63 give the
    unnormalized attention output, rows 64-127 the softmax denominator Z
    replicated, so normalization is a plain DVE multiply.
  - PSUM budget (8 banks): scores tag [128,2,512]x2 = 4, avz [128,512]x2 = 2,
    shared qk/v/out-proj accumulator [128,512]x2 = 2.
"""

import numpy as np
import ml_dtypes

import concourse.bacc as bacc
import concourse.mybir as mybir
import concourse.tile as tile
from concourse.bass_utils import run_bass_kernel_spmd

P = 128
DIM = 1024
HEADS = 16
B = 2
N = 2048
NCORES = 8
HGROUPS = 4                     # head-groups (tensor parallel)
H_LOC = HEADS // HGROUPS        # 4 heads per core
DH = DIM // HEADS               # 64
F_LOC = H_LOC * DH              # 256 features per core (per q/k/v)
SCALE = DIM ** -0.5             # exactly 1/32

F32 = mybir.dt.float32
BF16 = mybir.dt.bfloat16
EXP = mybir.ActivationFunctionType.Exp
NPBF16 = ml_dtypes.bfloat16

IC = N // 512                   # query chunks of 512
JT = N // P                     # 16 key tiles of 128
NU = IC * 2                     # (chunk, head-pair) units


def build_nc(kt: int):
    """Build the single-core program (identical on all 8 cores).

    kt: number of 128-row contraction tiles for the qkv projection
        (8 for dim=1024, 9 when a ones-row block is appended to fold biases).
    """
    nc = bacc.Bacc(trn_type="TRN2")

    xT = nc.dram_tensor("xT", (kt * P, N), BF16, kind="ExternalInput")
    w = nc.dram_tensor("w", (kt * P, 3 * F_LOC), BF16, kind="ExternalInput")
    wo = nc.dram_tensor("wo", (F_LOC, DIM), BF16, kind="ExternalInput")
    out = nc.dram_tensor("out", (N, DIM), BF16, kind="ExternalOutput")

    xT_t = xT[:].rearrange("(t p) n -> p t n", p=P)        # [128, kt, N]
    w_t = w[:].rearrange("(t p) f -> p t f", p=P)          # [128, kt, 768]
    wo_t = wo[:].rearrange("(t p) e -> p t e", p=P)        # [128, 2, 1024]

    with tile.TileContext(nc) as tc:
        with (
            tc.tile_pool(name="persist", bufs=1) as persist,
            tc.tile_pool(name="esbp", bufs=18) as esbp,
            tc.tile_pool(name="normp", bufs=2) as normp,
            tc.tile_pool(name="outp", bufs=4) as outp,
            tc.tile_pool(name="psum", bufs=2, space="PSUM") as psum,
        ):
            x_sb = persist.tile([P, kt, N], BF16, tag="x")
            w_sb = persist.tile([P, kt, 3 * F_LOC], BF16, tag="w")
            qT = persist.tile([P, 2, N], BF16, tag="qT")     # [feat, ft, tok]
            kT = persist.tile([P, 2, N], BF16, tag="kT")
            # v interleaved with ones columns: slot 2h = v_h, slot 2h+1 = 1.0
            # so that lhsT = vo[:, jt, 2h:2h+2, :] is [v_h | ones] (M=128).
            vo = persist.tile([P, JT, 2 * H_LOC, DH], BF16, tag="vo")
            vT = persist.tile([P, 2, N], BF16, tag="vT")       # [vfeat, ft, tok]
            outT = persist.tile([P, 2, N], BF16, tag="outT")   # [hd, kp, tok]
            wo_sb = persist.tile([P, 2, DIM], BF16, tag="wo")

            # PE warmup: the HAM clock gate keeps the PE at 1.2 GHz until
            # ~3.4us of sustained activity.  Burn that window on junk
            # matmuls over memset tiles while the input DMAs run, and
            # trigger the exp table load (~2.7us) early.  These memsets
            # must precede the big vo memset on gpsimd or the warmup
            # stalls ~9us behind it.
            wml = persist.tile([P, P], BF16, tag="wml")
            wmr = persist.tile([P, 512], BF16, tag="wmr")
            nc.gpsimd.memset(wml, 1.0)
            nc.gpsimd.memset(wmr, 1.0)
            wme = esbp.tile([P, 2, 512], BF16, tag="e", name="warm_e")
            nc.scalar.activation(wme[:, 0, 0:8], wmr[:, 0:8], EXP)

            def warm_mms(n, label):
                for g in range((n + 4) // 5):
                    ps = psum.tile([P, 512], F32, tag="acc",
                                   name=f"warm_{label}_{g}")
                    for i in range(min(5, n - g * 5)):
                        nc.tensor.matmul(ps, lhsT=wml, rhs=wmr,
                                         start=(i == 0), stop=True,
                                         skip_group_check=True)

            warm_mms(12, "head")
            nc.gpsimd.memset(vo[:, :, 1::2, :], 1.0)

            # ---- input DMA, one batched transfer per section -----------
            def dma_w(c0, c1):
                nc.sync.dma_start(out=w_sb[:, :, c0:c1], in_=w_t[:, :, c0:c1])

            def dma_x(c):
                csl = slice(c * 512, (c + 1) * 512)
                nc.sync.dma_start(out=x_sb[:, :, csl], in_=xT_t[:, :, csl])

            dma_w(F_LOC, 2 * F_LOC)          # k columns
            dma_x(0)
            dma_w(0, F_LOC)                  # q columns (pre-scaled)
            dma_x(1)
            dma_x(2)
            dma_x(3)
            dma_w(2 * F_LOC, 3 * F_LOC)      # v columns
            nc.sync.dma_start(out=wo_sb, in_=wo_t)

            # ---- PE work generators ------------------------------------
            def qk_group(which, ft, c):
                """q/k projection: one [128 feat, 512 tok] accumulation."""
                csl = slice(c * 512, (c + 1) * 512)
                f0 = which * F_LOC + ft * P
                ps = psum.tile([P, 512], F32, tag="acc",
                               name=f"qk{which}_{ft}_{c}")
                for k in range(kt):
                    nc.tensor.matmul(
                        ps,
                        lhsT=w_sb[:, k, f0:f0 + P],
                        rhs=x_sb[:, k, csl],
                        start=(k == 0),
                        stop=(k == kt - 1),
                        skip_group_check=True,
                    )
                dst = qT if which == 0 else kT
                nc.vector.tensor_copy(dst[:, ft, csl], ps)

            def v_group(ft, c):
                """v projection, feature-major like q/k (N=512 avoids the
                token-major form's 2x LDWEIGHTS tax), then the idle DMA
                xbar transposes each head tile into token-major vo."""
                csl = slice(c * 512, (c + 1) * 512)
                f0 = 2 * F_LOC + ft * P
                ps = psum.tile([P, 512], F32, tag="acc", name=f"vf{ft}_{c}")
                for k in range(kt):
                    nc.tensor.matmul(
                        ps,
                        lhsT=w_sb[:, k, f0:f0 + P],
                        rhs=x_sb[:, k, csl],
                        start=(k == 0),
                        stop=(k == kt - 1),
                        skip_group_check=True,
                    )
                nc.vector.tensor_copy(vT[:, ft, csl], ps)
                for jt in range(4 * c, 4 * c + 4):
                    tsl = slice(jt * P, (jt + 1) * P)
                    for hp in range(2):
                        h = ft * 2 + hp
                        nc.sync.dma_start_transpose(
                            out=vo[:, jt, 2 * h, :],
                            in_=vT[hp * DH:(hp + 1) * DH, ft, tsl],
                        )

            def o_group(ic, it, ec):
                """output projection: [128 tok, 512 emb], K=256 (2 tiles)."""
                i0 = (ic * 4 + it) * P
                esl = slice(ec * 512, (ec + 1) * 512)
                po = psum.tile([P, 512], F32, tag="acc", name=f"po{ic}_{it}_{ec}")
                for kp in range(2):
                    nc.tensor.matmul(
                        po,
                        lhsT=outT[:, kp, i0:i0 + P],
                        rhs=wo_sb[:, kp, esl],
                        start=(kp == 0),
                        stop=(kp == 1),
                        skip_group_check=True,
                    )
                po_sb = outp.tile([P, 512], BF16, tag="po_sb",
                                  name=f"posb{ic}_{it}_{ec}")
                nc.vector.tensor_copy(po_sb, po)
                nc.sync.dma_start(out=out[i0:i0 + P, esl], in_=po_sb)

            # filler queue: drip-fed into the PE stream between attention
            # steps, throttled by an estimated PE-vs-ACT clock so the PE
            # stays just behind the exp stream (ACT must never starve for
            # scores). force() handles hard deadlines (deps of the next
            # attention step) regardless of budget.
            fillers = []                      # list of (key, fn, est_us, deps)
            emitted = set()
            clock = {"pe": 0.0, "act": 0.0}   # estimated engine timelines

            def push(key, fn, est, deps=()):
                fillers.append((key, fn, est, deps))

            def _emit(key, fn, est, deps):
                for dk in deps:
                    if dk not in emitted:
                        force(dk)
                emitted.add(key)
                clock["pe"] += est
                fn()

            def force(key):
                for i, (k2, fn, est, deps) in enumerate(fillers):
                    if k2 == key:
                        fillers.pop(i)
                        _emit(k2, fn, est, deps)
                        return
                assert key in emitted, f"missing filler {key}"

            def pop_budget(slack=0.3):
                while fillers and clock["pe"] < clock["act"] - slack:
                    key, fn, est, deps = fillers.pop(0)
                    _emit(key, fn, est, deps)

            def pop_all():
                while fillers:
                    key, fn, est, deps = fillers.pop(0)
                    _emit(key, fn, est, deps)

            QK_US, V_US, O_US = 3.0, 3.3, 0.85
            for c in range(1, IC):
                push(("k", 0, c), (lambda c=c: qk_group(1, 0, c)), QK_US)
            for c in range(2, IC):
                push(("k", 1, c), (lambda ft=1, c=c: qk_group(1, ft, c)), QK_US)
            push(("q", 1, 0), (lambda: qk_group(0, 1, 0)), QK_US)
            for c in range(IC):
                for ft in range(2):
                    push(("v", ft, c), (lambda ft=ft, c=c: v_group(ft, c)),
                         V_US)
            for c in range(1, IC):
                push(("q", 0, c), (lambda c=c: qk_group(0, 0, c)), QK_US)
                push(("q", 1, c), (lambda c=c: qk_group(0, 1, c)), QK_US)

            # ---- attention pipeline ------------------------------------
            # unit u = (ic, pr): scores+exp for u stream in unit u; attn@v
            # for u-1 rides along one unit behind (esb holds the lag).
            avz = {}                          # (u, h2) -> psum tile
            esb = {}                          # (u, jt) -> sbuf exp tile

            def scores_step(u, jt):
                ic, pr = divmod(u, 2)
                isl = slice(ic * 512, (ic + 1) * 512)
                ss = psum.tile([P, 2, 512], F32, tag="s", name=f"s{u}_{jt}")
                for h2 in range(2):
                    hsl = slice(h2 * DH, (h2 + 1) * DH)
                    nc.tensor.matmul(
                        ss[:, h2, :],
                        lhsT=kT[hsl, pr, jt * P:(jt + 1) * P],
                        rhs=qT[hsl, pr, isl],
                        start=True,
                        stop=True,
                        skip_group_check=True,
                    )
                # flat views: a 2-D free AP over PSUM costs ~+400ns/inst
                e = esbp.tile([P, 2, 512], BF16, tag="e", name=f"e{u}_{jt}")
                nc.scalar.activation(e.rearrange("p a b -> p (a b)"),
                                     ss.rearrange("p a b -> p (a b)"), EXP)
                esb[(u, jt)] = e

            def attn_step(u, jt):
                for h2 in range(2):
                    if jt == 0:
                        avz[(u, h2)] = psum.tile(
                            [P, 512], F32, tag="avz", name=f"avz{u}_{h2}")
                    h = (u % 2) * 2 + h2
                    nc.tensor.matmul(
                        avz[(u, h2)],
                        lhsT=vo[:, jt, 2 * h:2 * h + 2, :],
                        rhs=esb[(u, jt)][:, h2, :],
                        start=(jt == 0),
                        stop=(jt == JT - 1),
                        skip_group_check=True,
                    )

            def norm_unit(u, muls_on_gpsimd=False):
                """avz -> outT: copy out of PSUM, 1/Z, multiply (all DVE)."""
                ic, pr = divmod(u, 2)
                isl = slice(ic * 512, (ic + 1) * 512)
                azs = normp.tile([P, 2, 512], F32, tag="azs", name=f"azs{u}")
                for h2 in range(2):
                    nc.vector.tensor_copy(azs[:, h2, :], avz.pop((u, h2)))
                # reciprocal_approx_fast (custom DVE op) mishandles
                # partition-shifted in/out, so shift Z down to partitions
                # 0-63 with a plain copy first, then invert at same base.
                zt = normp.tile([DH, 2, 512], F32, tag="zt", name=f"zt{u}")
                nc.vector.tensor_copy(zt, azs[DH:P, :, :])
                rz = normp.tile([DH, 2, 512], F32, tag="rz", name=f"rz{u}")
                nc.vector.reciprocal_approx_fast(out=rz, in_=zt)
                eng = nc.gpsimd if muls_on_gpsimd else nc.vector
                for h2 in range(2):
                    eng.tensor_mul(
                        out=outT[h2 * DH:(h2 + 1) * DH, pr, isl],
                        in0=azs[0:DH, h2, :],
                        in1=rz[:, h2, :],
                    )
                for jt in range(JT):
                    del esb[(u, jt)]

            # head: first k/q chunks, two primed scores (starts the exp
            # stream), then the second k-feature-half chunks while ACT
            # chews.  scores(u0,jt) only needs kproj chunk jt//4, so the
            # rest of the k projection streams inside unit 0 just before
            # the scores that consume it.
            qk_group(1, 0, 0)
            emitted.add(("k", 0, 0))
            qk_group(0, 0, 0)
            emitted.add(("q", 0, 0))
            clock["pe"] = 2 * QK_US
            clock["act"] = clock["pe"] + 3.0   # table load + first scores
            scores_step(0, 0)
            scores_step(0, 1)
            clock["pe"] += 0.8
            clock["act"] += 2.2
            qk_group(1, 1, 0)
            emitted.add(("k", 1, 0))
            qk_group(1, 1, 1)
            emitted.add(("k", 1, 1))
            clock["pe"] += 2 * QK_US

            for u in range(NU):
                ic, pr = divmod(u, 2)
                # safety net: hard deps of this unit's scores
                if ("q", pr, ic) not in emitted:
                    force(("q", pr, ic))
                for jt in range(2 if u == 0 else 0, JT):
                    # jt-granular k-projection deadlines: feature half pr,
                    # chunk jt//4 is all that scores(u,jt) needs.
                    if ic == 0 and jt % 4 == 0 and jt > 0:
                        if ("k", pr, jt // 4) not in emitted:
                            force(("k", pr, jt // 4))
                    if u > 0:
                        # attn first: its deps (exp of unit u-1) cleared a
                        # whole unit ago, so it streams while scores(u,jt)
                        # waits out its PSUM-bank rotation.
                        for ft in range(2):
                            if ("v", ft, jt // 4) not in emitted:
                                force(("v", ft, jt // 4))
                        attn_step(u - 1, jt)
                        clock["pe"] += 0.54
                    scores_step(u, jt)
                    clock["pe"] += 0.4
                    clock["act"] = max(clock["act"], clock["pe"]) + 1.11
                    if jt == 10 and u + 1 < NU:
                        # pre-emit the next unit's q projection so it
                        # doesn't stall the exp stream at the boundary
                        icn, prn = divmod(u + 1, 2)
                        if ("q", prn, icn) not in emitted:
                            force(("q", prn, icn))
                    pop_budget()
                if u > 0:
                    norm_unit(u - 1)
                    if u % 2 == 0:
                        icd = (u - 2) // 2
                        for it in range(4):
                            for ec in range(2):
                                push(("o", icd, it, ec),
                                     (lambda ic=icd, it=it, ec=ec:
                                      o_group(ic, it, ec)), O_US)

            # drain: last unit's attn@v (overlaps the final exps), norm,
            # out-proj; leftover fillers first so they overlap exp too.
            pop_all()
            for jt in range(JT):
                attn_step(NU - 1, jt)
            norm_unit(NU - 1, muls_on_gpsimd=True)
            warm_mms(12, "tail")     # keep the PE clock up through norm
            for it in range(4):
                for ec in range(2):
                    o_group(IC - 1, it, ec)
    nc.finalize()
    return nc


def _shard_inputs(x, w_qkv, b_qkv, w_out):
    """Host-side sharding: per-core input dicts (see module docstring)."""
    x = np.ascontiguousarray(x, dtype=np.float32)
    w_qkv = np.asarray(w_qkv, dtype=np.float32)
    b_qkv = np.asarray(b_qkv, dtype=np.float32)
    w_out = np.asarray(w_out, dtype=np.float32)

    has_bias = bool(np.any(b_qkv))
    kt = DIM // P + (1 if has_bias else 0)

    in_maps = []
    for c in range(NCORES):
        b = c // HGROUPS
        hg = c % HGROUPS
        fsl = slice(hg * F_LOC, (hg + 1) * F_LOC)
        # per-core weight shard [dim, 768]: q (pre-scaled), k, v columns
        w_shard = np.concatenate(
            [
                w_qkv[:, 0 * DIM:1 * DIM][:, fsl] * SCALE,
                w_qkv[:, 1 * DIM:2 * DIM][:, fsl],
                w_qkv[:, 2 * DIM:3 * DIM][:, fsl],
            ],
            axis=1,
        )
        xT_aug = np.zeros((kt * P, N), dtype=np.float32)
        xT_aug[:DIM] = x[b].T
        w_aug = np.zeros((kt * P, 3 * F_LOC), dtype=np.float32)
        w_aug[:DIM] = w_shard
        if has_bias:
            xT_aug[DIM] = 1.0
            w_aug[DIM] = np.concatenate(
                [
                    b_qkv[0 * DIM:1 * DIM][fsl] * SCALE,
                    b_qkv[1 * DIM:2 * DIM][fsl],
                    b_qkv[2 * DIM:3 * DIM][fsl],
                ]
            )
        in_maps.append(
            {
                "xT": np.ascontiguousarray(xT_aug).astype(NPBF16),
                "w": np.ascontiguousarray(w_aug).astype(NPBF16),
                "wo": np.ascontiguousarray(w_out[fsl, :]).astype(NPBF16),
            }
        )
    return in_maps, kt


def _run(x, w_qkv, b_qkv, b_out, w_out, trace=False, **spmd_kwargs):
    in_maps, kt = _shard_inputs(x, w_qkv, b_qkv, w_out)
    nc = build_nc(kt)
    res = run_bass_kernel_spmd(
        nc, in_maps, core_ids=list(range(NCORES)), trace=trace, **spmd_kwargs
    )
    b_out = np.asarray(b_out, dtype=np.float32)
    full = np.empty((B, N, DIM), dtype=np.float32)
    for b in range(B):
        acc = res.results[b * HGROUPS]["out"].astype(np.float32)
        for hg in range(1, HGROUPS):
            acc = acc + res.results[b * HGROUPS + hg]["out"].astype(np.float32)
        full[b] = acc + b_out
    return full, res


def kernel(x, w_qkv, b_qkv, w_out, b_out):
    full, _ = _run(x, w_qkv, b_qkv, b_out, w_out, trace=False)
    return full


# revision 36
# speedup vs baseline: 1.1801x; 1.1801x over previous
"""Trainium2 Bass kernel: multi-head self-attention block (dense transformer).

Reference computation (fp32):
    qkv = x @ w_qkv + b_qkv                  # x [b, n, dim], w_qkv [dim, 3*dim]
    q, k, v = split(qkv); heads = 16, dh = 64
    dots = (q @ k^T) * dim**-0.5  (per head)
    attn = softmax(dots, axis=-1)
    out  = (attn @ v) @ w_out + b_out        # [b, n, dim]

Sharding (8 cores): data-parallel over batch (b=2) x tensor-parallel over
head-groups (4 groups of 4 heads).  core c -> batch c//4, head-group c%4.
Each core computes q/k/v for its 4 heads only, runs attention, and multiplies
by its 256-row slice of w_out, producing a partial [n, dim] output.  The host
sums the 4 partials per batch (the "all-reduce") and adds b_out.

v2 design (bf16 + engine-balance; ~2x over the fp32r v1):
  - ALL matmul operands are bf16 (fp32 PSUM accumulate).  fp32r pays an
    unhidden ~107ns LDWEIGHTS per matmul (no FWL for fp32); bf16 enables
    FWL so an N=512 matmul costs ~216ns vs ~330ns fp32r.  End-to-end bf16
    keeps global rel err ~5e-3 (gate 2e-2).
  - the scalar engine (ACT) runs exp at 1 elem/cycle/lane @1.2GHz: 16.8M
    score elements/core = ~110us busy -- a hard floor that rivals the PE
    (~140us).  So: ACT does ONLY exp (one FD=1024 instruction per key-tile,
    both heads of the pair in one [128,2,512] PSUM tile); every PSUM->SBUF
    copy runs on DVE; normalization uses reciprocal_approx_fast (5x faster
    than DVE reciprocal, 18-bit accurate) once per (chunk, pair).
  - software pipeline ACROSS the whole kernel, not per-phase: after the
    k-projection and first q-chunk land, scores+exp stream continuously;
    v-projection, remaining q-projection and the output projection are
    drip-fed into the PE stream as filler so the PE never idles while ACT
    works; attn@v trails exp by one (chunk, pair) unit (deep esb buffer).
  - scores are computed TRANSPOSED (S^T [j, i]) so attn@v needs no
    transpose; attn@v stationary = [v_h | ones] (M=128): rows 0-63 give the
    unnormalized attention output, rows 64-127 the softmax denominator Z
    replicated, so normalization is a plain DVE multiply.
  - PSUM budget (8 banks): scores tag [128,2,512]x2 = 4, avz [128,512]x2 = 2,
    shared qk/v/out-proj accumulator [128,512]x2 = 2.
"""

import numpy as np
import ml_dtypes

import concourse.bacc as bacc
import concourse.mybir as mybir
import concourse.tile as tile
from concourse.bass_utils import run_bass_kernel_spmd

P = 128
DIM = 1024
HEADS = 16
B = 2
N = 2048
NCORES = 8
HGROUPS = 4                     # head-groups (tensor parallel)
H_LOC = HEADS // HGROUPS        # 4 heads per core
DH = DIM // HEADS               # 64
F_LOC = H_LOC * DH              # 256 features per core (per q/k/v)
SCALE = DIM ** -0.5             # exactly 1/32

F32 = mybir.dt.float32
BF16 = mybir.dt.bfloat16
EXP = mybir.ActivationFunctionType.Exp
NPBF16 = ml_dtypes.bfloat16

IC = N // 512                   # query chunks of 512
JT = N // P                     # 16 key tiles of 128
NU = IC * 2                     # (chunk, head-pair) units


def build_nc(kt: int):
    """Build the single-core program (identical on all 8 cores).

    kt: number of 128-row contraction tiles for the qkv projection
        (8 for dim=1024, 9 when a ones-row block is appended to fold biases).
    """
    nc = bacc.Bacc(trn_type="TRN2")

    xT = nc.dram_tensor("xT", (kt * P, N), BF16, kind="ExternalInput")
    w = nc.dram_tensor("w", (kt * P, 3 * F_LOC), BF16, kind="ExternalInput")
    wo = nc.dram_tensor("wo", (F_LOC, DIM), BF16, kind="ExternalInput")
    out = nc.dram_tensor("out", (N, DIM), BF16, kind="ExternalOutput")

    xT_t = xT[:].rearrange("(t p) n -> p t n", p=P)        # [128, kt, N]
    w_t = w[:].rearrange("(t p) f -> p t f", p=P)          # [128, kt, 768]
    wo_t = wo[:].rearrange("(t p) e -> p t e", p=P)        # [128, 2, 1024]

    with tile.TileContext(nc) as tc:
        with (
            tc.tile_pool(name="persist", bufs=1) as persist,
            tc.tile_pool(name="esbp", bufs=18) as esbp,
            tc.tile_pool(name="normp", bufs=2) as normp,
            tc.tile_pool(name="outp", bufs=4) as outp,
            tc.tile_pool(name="psum", bufs=2, space="PSUM") as psum,
        ):
            x_sb = persist.tile([P, kt, N], BF16, tag="x")
            w_sb = persist.tile([P, kt, 3 * F_LOC], BF16, tag="w")
            qT = persist.tile([P, 2, N], BF16, tag="qT")     # [feat, ft, tok]
            kT = persist.tile([P, 2, N], BF16, tag="kT")
            # v interleaved with ones columns: slot 2h = v_h, slot 2h+1 = 1.0
            # so that lhsT = vo[:, jt, 2h:2h+2, :] is [v_h | ones] (M=128).
            vo = persist.tile([P, JT, 2 * H_LOC, DH], BF16, tag="vo")
            outT = persist.tile([P, 2, N], BF16, tag="outT")   # [hd, kp, tok]
            wo_sb = persist.tile([P, 2, DIM], BF16, tag="wo")

            # PE warmup: the HAM clock gate keeps the PE at 1.2 GHz until
            # ~3.4us of sustained activity.  Burn that window on junk
            # matmuls over memset tiles while the input DMAs run, and
            # trigger the exp table load (~2.7us) early.  These memsets
            # must precede the big vo memset on gpsimd or the warmup
            # stalls ~9us behind it.
            wml = persist.tile([P, P], BF16, tag="wml")
            wmr = persist.tile([P, 512], BF16, tag="wmr")
            nc.gpsimd.memset(wml, 1.0)
            nc.gpsimd.memset(wmr, 1.0)
            wme = esbp.tile([P, 2, 512], BF16, tag="e", name="warm_e")
            nc.scalar.activation(wme[:, 0, 0:8], wmr[:, 0:8], EXP)

            def warm_mms(n, label):
                for g in range((n + 4) // 5):
                    ps = psum.tile([P, 512], F32, tag="acc",
                                   name=f"warm_{label}_{g}")
                    for i in range(min(5, n - g * 5)):
                        nc.tensor.matmul(ps, lhsT=wml, rhs=wmr,
                                         start=(i == 0), stop=True,
                                         skip_group_check=True)

            warm_mms(12, "head")
            nc.gpsimd.memset(vo[:, :, 1::2, :], 1.0)

            # ---- input DMA, one batched transfer per section -----------
            def dma_w(c0, c1):
                nc.sync.dma_start(out=w_sb[:, :, c0:c1], in_=w_t[:, :, c0:c1])

            def dma_x(c):
                csl = slice(c * 512, (c + 1) * 512)
                nc.sync.dma_start(out=x_sb[:, :, csl], in_=xT_t[:, :, csl])

            dma_w(F_LOC, 2 * F_LOC)          # k columns
            dma_x(0)
            dma_w(0, F_LOC)                  # q columns (pre-scaled)
            dma_x(1)
            dma_x(2)
            dma_x(3)
            dma_w(2 * F_LOC, 3 * F_LOC)      # v columns
            nc.sync.dma_start(out=wo_sb, in_=wo_t)

            # ---- PE work generators ------------------------------------
            def qk_group(which, ft, c):
                """q/k projection: one [128 feat, 512 tok] accumulation."""
                csl = slice(c * 512, (c + 1) * 512)
                f0 = which * F_LOC + ft * P
                ps = psum.tile([P, 512], F32, tag="acc",
                               name=f"qk{which}_{ft}_{c}")
                for k in range(kt):
                    nc.tensor.matmul(
                        ps,
                        lhsT=w_sb[:, k, f0:f0 + P],
                        rhs=x_sb[:, k, csl],
                        start=(k == 0),
                        stop=(k == kt - 1),
                        skip_group_check=True,
                    )
                dst = qT if which == 0 else kT
                nc.vector.tensor_copy(dst[:, ft, csl], ps)

            def v_group(jt):
                """v projection: one [128 tok, 256 vfeat] accumulation."""
                tsl = slice(jt * P, (jt + 1) * P)
                ps = psum.tile([P, 512], F32, tag="acc", name=f"v{jt}")
                for k in range(kt):
                    nc.tensor.matmul(
                        ps[:, 0:F_LOC],
                        lhsT=x_sb[:, k, tsl],
                        rhs=w_sb[:, k, 2 * F_LOC:3 * F_LOC],
                        start=(k == 0),
                        stop=(k == kt - 1),
                        skip_group_check=True,
                    )
                nc.vector.tensor_copy(vo[:, jt, 0::2, :], ps[:, 0:F_LOC])

            def o_group(ic, it, ec):
                """output projection: [128 tok, 512 emb], K=256 (2 tiles)."""
                i0 = (ic * 4 + it) * P
                esl = slice(ec * 512, (ec + 1) * 512)
                po = psum.tile([P, 512], F32, tag="acc", name=f"po{ic}_{it}_{ec}")
                for kp in range(2):
                    nc.tensor.matmul(
                        po,
                        lhsT=outT[:, kp, i0:i0 + P],
                        rhs=wo_sb[:, kp, esl],
                        start=(kp == 0),
                        stop=(kp == 1),
                        skip_group_check=True,
                    )
                po_sb = outp.tile([P, 512], BF16, tag="po_sb",
                                  name=f"posb{ic}_{it}_{ec}")
                nc.vector.tensor_copy(po_sb, po)
                nc.sync.dma_start(out=out[i0:i0 + P, esl], in_=po_sb)

            # filler queue: drip-fed into the PE stream between attention
            # steps, throttled by an estimated PE-vs-ACT clock so the PE
            # stays just behind the exp stream (ACT must never starve for
            # scores). force() handles hard deadlines (deps of the next
            # attention step) regardless of budget.
            fillers = []                      # list of (key, fn, est_us, deps)
            emitted = set()
            clock = {"pe": 0.0, "act": 0.0}   # estimated engine timelines

            def push(key, fn, est, deps=()):
                fillers.append((key, fn, est, deps))

            def _emit(key, fn, est, deps):
                for dk in deps:
                    if dk not in emitted:
                        force(dk)
                emitted.add(key)
                clock["pe"] += est
                fn()

            def force(key):
                for i, (k2, fn, est, deps) in enumerate(fillers):
                    if k2 == key:
                        fillers.pop(i)
                        _emit(k2, fn, est, deps)
                        return
                assert key in emitted, f"missing filler {key}"

            def pop_budget(slack=0.3):
                while fillers and clock["pe"] < clock["act"] - slack:
                    key, fn, est, deps = fillers.pop(0)
                    _emit(key, fn, est, deps)

            def pop_all():
                while fillers:
                    key, fn, est, deps = fillers.pop(0)
                    _emit(key, fn, est, deps)

            QK_US, V_US, O_US = 3.0, 2.2, 0.85
            for c in range(1, IC):
                push(("k", 0, c), (lambda c=c: qk_group(1, 0, c)), QK_US)
            for c in range(2, IC):
                push(("k", 1, c), (lambda ft=1, c=c: qk_group(1, ft, c)), QK_US)
            push(("q", 1, 0), (lambda: qk_group(0, 1, 0)), QK_US)
            for jt in range(JT):
                push(("v", jt), (lambda jt=jt: v_group(jt)), V_US)
            for c in range(1, IC):
                push(("q", 0, c), (lambda c=c: qk_group(0, 0, c)), QK_US)
                push(("q", 1, c), (lambda c=c: qk_group(0, 1, c)), QK_US)

            # ---- attention pipeline ------------------------------------
            # unit u = (ic, pr): scores+exp for u stream in unit u; attn@v
            # for u-1 rides along one unit behind (esb holds the lag).
            avz = {}                          # (u, h2) -> psum tile
            esb = {}                          # (u, jt) -> sbuf exp tile

            def scores_step(u, jt):
                ic, pr = divmod(u, 2)
                isl = slice(ic * 512, (ic + 1) * 512)
                ss = psum.tile([P, 2, 512], F32, tag="s", name=f"s{u}_{jt}")
                for h2 in range(2):
                    hsl = slice(h2 * DH, (h2 + 1) * DH)
                    nc.tensor.matmul(
                        ss[:, h2, :],
                        lhsT=kT[hsl, pr, jt * P:(jt + 1) * P],
                        rhs=qT[hsl, pr, isl],
                        start=True,
                        stop=True,
                        skip_group_check=True,
                    )
                # flat views: a 2-D free AP over PSUM costs ~+400ns/inst
                e = esbp.tile([P, 2, 512], BF16, tag="e", name=f"e{u}_{jt}")
                nc.scalar.activation(e.rearrange("p a b -> p (a b)"),
                                     ss.rearrange("p a b -> p (a b)"), EXP)
                esb[(u, jt)] = e

            def attn_step(u, jt):
                for h2 in range(2):
                    if jt == 0:
                        avz[(u, h2)] = psum.tile(
                            [P, 512], F32, tag="avz", name=f"avz{u}_{h2}")
                    h = (u % 2) * 2 + h2
                    nc.tensor.matmul(
                        avz[(u, h2)],
                        lhsT=vo[:, jt, 2 * h:2 * h + 2, :],
                        rhs=esb[(u, jt)][:, h2, :],
                        start=(jt == 0),
                        stop=(jt == JT - 1),
                        skip_group_check=True,
                    )

            def norm_unit(u, muls_on_gpsimd=False):
                """avz -> outT: copy out of PSUM, 1/Z, multiply (all DVE)."""
                ic, pr = divmod(u, 2)
                isl = slice(ic * 512, (ic + 1) * 512)
                azs = normp.tile([P, 2, 512], F32, tag="azs", name=f"azs{u}")
                for h2 in range(2):
                    nc.vector.tensor_copy(azs[:, h2, :], avz.pop((u, h2)))
                # reciprocal_approx_fast (custom DVE op) mishandles
                # partition-shifted in/out, so shift Z down to partitions
                # 0-63 with a plain copy first, then invert at same base.
                zt = normp.tile([DH, 2, 512], F32, tag="zt", name=f"zt{u}")
                nc.vector.tensor_copy(zt, azs[DH:P, :, :])
                rz = normp.tile([DH, 2, 512], F32, tag="rz", name=f"rz{u}")
                nc.vector.reciprocal_approx_fast(out=rz, in_=zt)
                eng = nc.gpsimd if muls_on_gpsimd else nc.vector
                for h2 in range(2):
                    eng.tensor_mul(
                        out=outT[h2 * DH:(h2 + 1) * DH, pr, isl],
                        in0=azs[0:DH, h2, :],
                        in1=rz[:, h2, :],
                    )
                for jt in range(JT):
                    del esb[(u, jt)]

            # head: first k/q chunks, two primed scores (starts the exp
            # stream), then the second k-feature-half chunks while ACT
            # chews.  scores(u0,jt) only needs kproj chunk jt//4, so the
            # rest of the k projection streams inside unit 0 just before
            # the scores that consume it.
            qk_group(1, 0, 0)
            emitted.add(("k", 0, 0))
            qk_group(0, 0, 0)
            emitted.add(("q", 0, 0))
            clock["pe"] = 2 * QK_US
            clock["act"] = clock["pe"] + 3.0   # table load + first scores
            scores_step(0, 0)
            scores_step(0, 1)
            clock["pe"] += 0.8
            clock["act"] += 2.2
            qk_group(1, 1, 0)
            emitted.add(("k", 1, 0))
            qk_group(1, 1, 1)
            emitted.add(("k", 1, 1))
            clock["pe"] += 2 * QK_US

            for u in range(NU):
                ic, pr = divmod(u, 2)
                # safety net: hard deps of this unit's scores
                if ("q", pr, ic) not in emitted:
                    force(("q", pr, ic))
                for jt in range(2 if u == 0 else 0, JT):
                    # jt-granular k-projection deadlines: feature half pr,
                    # chunk jt//4 is all that scores(u,jt) needs.
                    if ic == 0 and jt % 4 == 0 and jt > 0:
                        if ("k", pr, jt // 4) not in emitted:
                            force(("k", pr, jt // 4))
                    if u > 0:
                        # attn first: its deps (exp of unit u-1) cleared a
                        # whole unit ago, so it streams while scores(u,jt)
                        # waits out its PSUM-bank rotation.
                        if ("v", jt) not in emitted:
                            force(("v", jt))
                        attn_step(u - 1, jt)
                        clock["pe"] += 0.54
                    scores_step(u, jt)
                    clock["pe"] += 0.4
                    clock["act"] = max(clock["act"], clock["pe"]) + 1.11
                    if jt == 10 and u + 1 < NU:
                        # pre-emit the next unit's q projection so it
                        # doesn't stall the exp stream at the boundary
                        icn, prn = divmod(u + 1, 2)
                        if ("q", prn, icn) not in emitted:
                            force(("q", prn, icn))
                    pop_budget()
                if u > 0:
                    norm_unit(u - 1)
                    if u % 2 == 0:
                        icd = (u - 2) // 2
                        for it in range(4):
                            for ec in range(2):
                                push(("o", icd, it, ec),
                                     (lambda ic=icd, it=it, ec=ec:
                                      o_group(ic, it, ec)), O_US)

            # drain: last unit's attn@v (overlaps the final exps), norm,
            # out-proj; leftover fillers first so they overlap exp too.
            pop_all()
            for jt in range(JT):
                attn_step(NU - 1, jt)
            norm_unit(NU - 1, muls_on_gpsimd=True)
            warm_mms(12, "tail")     # keep the PE clock up through norm
            for it in range(4):
                for ec in range(2):
                    o_group(IC - 1, it, ec)
    nc.finalize()
    return nc


def _shard_inputs(x, w_qkv, b_qkv, w_out):
    """Host-side sharding: per-core input dicts (see module docstring)."""
    x = np.ascontiguousarray(x, dtype=np.float32)
    w_qkv = np.asarray(w_qkv, dtype=np.float32)
    b_qkv = np.asarray(b_qkv, dtype=np.float32)
    w_out = np.asarray(w_out, dtype=np.float32)

    has_bias = bool(np.any(b_qkv))
    kt = DIM // P + (1 if has_bias else 0)

    in_maps = []
    for c in range(NCORES):
        b = c // HGROUPS
        hg = c % HGROUPS
        fsl = slice(hg * F_LOC, (hg + 1) * F_LOC)
        # per-core weight shard [dim, 768]: q (pre-scaled), k, v columns
        w_shard = np.concatenate(
            [
                w_qkv[:, 0 * DIM:1 * DIM][:, fsl] * SCALE,
                w_qkv[:, 1 * DIM:2 * DIM][:, fsl],
                w_qkv[:, 2 * DIM:3 * DIM][:, fsl],
            ],
            axis=1,
        )
        xT_aug = np.zeros((kt * P, N), dtype=np.float32)
        xT_aug[:DIM] = x[b].T
        w_aug = np.zeros((kt * P, 3 * F_LOC), dtype=np.float32)
        w_aug[:DIM] = w_shard
        if has_bias:
            xT_aug[DIM] = 1.0
            w_aug[DIM] = np.concatenate(
                [
                    b_qkv[0 * DIM:1 * DIM][fsl] * SCALE,
                    b_qkv[1 * DIM:2 * DIM][fsl],
                    b_qkv[2 * DIM:3 * DIM][fsl],
                ]
            )
        in_maps.append(
            {
                "xT": np.ascontiguousarray(xT_aug).astype(NPBF16),
                "w": np.ascontiguousarray(w_aug).astype(NPBF16),
                "wo": np.ascontiguousarray(w_out[fsl, :]).astype(NPBF16),
            }
        )
    return in_maps, kt


def _run(x, w_qkv, b_qkv, b_out, w_out, trace=False, **spmd_kwargs):
    in_maps, kt = _shard_inputs(x, w_qkv, b_qkv, w_out)
    nc = build_nc(kt)
    res = run_bass_kernel_spmd(
        nc, in_maps, core_ids=list(range(NCORES)), trace=trace, **spmd_kwargs
    )
    b_out = np.asarray(b_out, dtype=np.float32)
    full = np.empty((B, N, DIM), dtype=np.float32)
    for b in range(B):
        acc = res.results[b * HGROUPS]["out"].astype(np.float32)
        for hg in range(1, HGROUPS):
            acc = acc + res.results[b * HGROUPS + hg]["out"].astype(np.float32)
        full[b] = acc + b_out
    return full, res


def kernel(x, w_qkv, b_qkv, w_out, b_out):
    full, _ = _run(x, w_qkv, b_qkv, b_out, w_out, trace=False)
    return full
